# revision 4
# baseline (speedup 1.0000x reference)
"""Trainium2 Bass kernel for nn_IntensityLoss (bilateral-filter intensity loss).

Math (all window sums use raw r_weights; the 1/25 normalizations cancel):
  A  = sum_t w_t                (25-tap sum, per pixel)
  Bf = sum_t fake_t  w_t ; Cf = sum_t fake_t^2  w_t   (taps = 5x5 shifted copies)
  Bg, Cg  likewise for gamma_hdr
  Bh = sum_t H_t w_t  with  H = hdr_original_im ** (1 - f)   (zero-padded)
  Vx  = max(Cx*A - Bx^2, 0) + eps*A^2        (= A^2 * (var + eps))
  num = K * sqrt(Vg) * (Bh + eps*A)          (K = gray_max / f)
  den = A * sqrt(Vf) + num
  r   = num / den                            (= 1 - std_fake/(std_fake+std_obj))
  out = sum(r * (A-1)) / sum(A-1)            (global over B*H*W pixels)

Sharding: core c handles batch b=c//2, rows [256*(c%2), +256).  Each core pads
to 275 "virtual" rows (11 chunks x 25 rows); pad rows get w=0.04 so A~=1,
w_blf=A-1~0 -> no contribution.

Layout: "diagonal stack" [125 partitions = 5 row-shifts x 25 rows, 512 cols].
All inputs are host-cast to bf16.  DVE forms all window products with fused
3D-AP bf16 muls (2x mode), PE reduces taps with sparse selector matmuls into
two 3-bank PSUM groups, ScalarE evacuates PSUM / squares / sqrts, GpSimd does
two epilogue muls, and the epilogue is pipelined per 125-row super-chunk.
"""

import sys

sys.path.insert(0, "/opt/trn_rl_repo")

import numpy as np
import ml_dtypes

import concourse.bass as bass
import concourse.bacc as bacc
import concourse.tile as tile
from concourse import mybir
from concourse.bass_utils import run_bass_kernel_spmd

F32 = mybir.dt.float32
BF16 = mybir.dt.bfloat16
AF = mybir.ActivationFunctionType
ALU = mybir.AluOpType
AX = mybir.AxisListType

EPS = 1e-5
H_IMG = 512
W_IMG = 512
B_SZ = 4
N_CORES = 8
RPC = 256          # real rows per core
QR = 25            # rows per chunk
NCH = 11           # chunks per core (275 virtual rows)
VROWS = NCH * QR   # 275
PROWS = 280        # padded image rows staged per core
PCOLS = 516        # padded image cols
FGC = 2 * PCOLS    # 1032, f+g packed row
WPAD = 0.04        # weight value for virtual-pad rows (A ~= 1)

_CACHE = {}


def _build_nc():
    nc = bacc.Bacc(None)
    wslab = nc.declare_dram_parameter("wslab", [5, VROWS, 5, W_IMG], BF16, isOutput=False)
    imfg = nc.declare_dram_parameter("imfg", [PROWS, 2, PCOLS], BF16, isOutput=False)
    imh = nc.declare_dram_parameter("imh", [PROWS, PCOLS], BF16, isOutput=False)
    hmask = nc.declare_dram_parameter("hmask", [PROWS, 1], F32, isOutput=False)
    gray = nc.declare_dram_parameter("gray", [H_IMG, W_IMG], BF16, isOutput=False)
    scal = nc.declare_dram_parameter("scal", [1, 4], F32, isOutput=False)
    stat = nc.declare_dram_parameter("stat", [5, 125, 125], BF16, isOutput=False)
    out = nc.declare_dram_parameter("out", [125, 2], F32, isOutput=True)

    himg = nc.dram_tensor("himg", [PROWS, PCOLS], BF16)

    HWT = VROWS * 5 * W_IMG  # a-shift stride in wslab

    with tile.TileContext(nc) as tc:
        with (
            tc.tile_pool(name="singles", bufs=1) as singles,
            tc.tile_pool(name="prep", bufs=2) as prep,
            tc.tile_pool(name="chunk", bufs=2) as chunk,
            tc.tile_pool(name="prod", bufs=2) as prod,
            tc.tile_pool(name="gstp", bufs=2) as gstp,
            tc.tile_pool(name="epi", bufs=2) as epi,
            tc.tile_pool(name="psAB", bufs=1, space="PSUM") as psum_stats,
            tc.tile_pool(name="psM", bufs=1, space="PSUM") as psum_misc,
        ):
            # ---------- phase 0 (scalar queue): selectors, scalars, H, gray ----------
            # selectors first: they gate every chunk matmul
            st_all = singles.tile([125, 5, 125], BF16)
            nc.scalar.dma_start(
                out=st_all[:],
                in_=bass.AP(
                    tensor=stat,
                    offset=0,
                    ap=[[125, 125], [125 * 125, 5], [1, 125]],
                ),
            )
            sc = singles.tile([1, 4], F32)
            nc.scalar.dma_start(out=sc[:], in_=scal[:])

            ones = singles.tile([1, 128], F32)
            nc.vector.memset(ones[:], 1.0)

            # broadcast 1-f and 1/f to all partitions via PE
            f1m_bc = singles.tile([128, 1], F32)
            finv_bc = singles.tile([128, 1], F32)
            ps_bc = psum_misc.tile([128, 1], F32, tag="bc")
            nc.tensor.matmul(ps_bc[:], ones[:], sc[0:1, 0:1], start=True, stop=True)
            nc.scalar.copy(f1m_bc[:], ps_bc[:])
            ps_bc2 = psum_misc.tile([128, 1], F32, tag="bc", name="ps_bc2")
            nc.tensor.matmul(ps_bc2[:], ones[:], sc[0:1, 1:2], start=True, stop=True)
            nc.scalar.copy(finv_bc[:], ps_bc2[:])

            # H = (hdr ** (1-f)) with zero padding, stored to DRAM in bf16
            row_tiles = [(0, 128), (128, 128), (256, PROWS - 256)]
            for r0, p in row_tiles:
                ht = prep.tile([128, PCOLS], BF16, tag="ht")
                nc.scalar.dma_start(out=ht[:p, :], in_=imh[r0 : r0 + p, :])
                lt = prep.tile([128, PCOLS], F32, tag="lt")
                nc.scalar.activation(lt[:p, :], ht[:p, :], AF.Ln)
                et = prep.tile([128, PCOLS], BF16, tag="et")
                nc.scalar.activation(et[:p, :], lt[:p, :], AF.Exp, scale=f1m_bc[:p, :])
                hm = prep.tile([128, 1], F32, tag="hm")
                nc.scalar.dma_start(out=hm[:p, :], in_=hmask[r0 : r0 + p, :])
                nc.vector.tensor_scalar_mul(et[:p, :], et[:p, :], hm[:p, 0:1])
                nc.vector.memset(et[:p, 0:2], 0.0)
                nc.vector.memset(et[:p, 514:516], 0.0)
                nc.scalar.dma_start(out=himg[r0 : r0 + p, :], in_=et[:p, :])

            # gray max over the full batch image
            gt = prep.tile([128, 2048], BF16)
            nc.scalar.dma_start(
                out=gt[:],
                in_=bass.AP(tensor=gray, offset=0, ap=[[2048, 128], [1, 2048]]),
            )
            gm = singles.tile([128, 1], F32)
            nc.vector.tensor_reduce(gm[:], gt[:], axis=AX.X, op=ALU.max)
            gmr = singles.tile([1, 128], F32)
            nc.scalar.dma_start(out=gmr[:], in_=gm[:])
            gms = singles.tile([1, 1], F32)
            nc.vector.tensor_reduce(gms[:], gmr[:], axis=AX.X, op=ALU.max)
            gm_bc = singles.tile([128, 1], F32)
            ps_bc3 = psum_misc.tile([128, 1], F32, tag="bc", name="ps_bc3")
            nc.tensor.matmul(ps_bc3[:], ones[:], gms[0:1, 0:1], start=True, stop=True)
            nc.scalar.copy(gm_bc[:], ps_bc3[:])
            k_sb = singles.tile([128, 1], F32)
            nc.vector.tensor_mul(k_sb[:], gm_bc[:], finv_bc[:])
            k2_sb = singles.tile([128, 1], F32)
            nc.vector.tensor_mul(k2_sb[:], k_sb[:], k_sb[:])

            # per-g accumulators
            accC = [singles.tile([125, 1], F32, name=f"accC{i}") for i in range(3)]
            accA = [singles.tile([125, 1], F32, name=f"accA{i}") for i in range(3)]

            psA = None
            psB = None

            # ---------- phase 1: chunks ----------
            for c in range(NCH):
                s = c % 5
                g = c // 5
                last_s = 4 if g < 2 else 0
                cr0 = c * QR

                # P: [125, 6 slots, 5 taps, 512]; slot 0 = w (DMA), 1=p1f,
                # 2=p2f, 3=p1g, 4=p2g, 5=p1h
                P = prod.tile([125, 6, 5, 512], BF16, tag="P")
                nc.sync.dma_start(
                    out=P[:, 0, :, :],
                    in_=bass.AP(
                        tensor=wslab,
                        offset=cr0 * 5 * W_IMG,
                        ap=[[HWT, 5], [5 * W_IMG, QR], [1, 5 * W_IMG]],
                    ),
                )
                fg0 = chunk.tile([125, 2, PCOLS], BF16, tag="fg0")
                nc.sync.dma_start(
                    out=fg0[:],
                    in_=bass.AP(
                        tensor=imfg,
                        offset=cr0 * FGC,
                        ap=[[FGC, 5], [FGC, QR], [1, FGC]],
                    ),
                )
                fg1 = chunk.tile([125, 1036], BF16, tag="fg1")
                nc.sync.dma_start(
                    out=fg1[:, 0:1031],
                    in_=bass.AP(
                        tensor=imfg,
                        offset=cr0 * FGC + 1,
                        ap=[[FGC, 5], [FGC, QR], [1, FGC - 1]],
                    ),
                )
                h0 = chunk.tile([125, PCOLS], BF16, tag="h0")
                nc.scalar.dma_start(
                    out=h0[:],
                    in_=bass.AP(
                        tensor=himg,
                        offset=cr0 * PCOLS,
                        ap=[[PCOLS, 5], [PCOLS, QR], [1, PCOLS]],
                    ),
                )
                h1 = chunk.tile([125, PCOLS], BF16, tag="h1")
                nc.scalar.dma_start(
                    out=h1[:, 0:515],
                    in_=bass.AP(
                        tensor=himg,
                        offset=cr0 * PCOLS + 1,
                        ap=[[PCOLS, 5], [PCOLS, QR], [1, PCOLS - 1]],
                    ),
                )

                # window views (overlapping 3D APs; inner step 1, even offsets)
                def win(t, base, stride_row, nb):
                    return bass.AP(
                        tensor=t.tensor,
                        offset=base,
                        ap=[[stride_row, 125], [2, nb], [1, 512]],
                    )

                f0w = win(fg0, 0, FGC, 3)
                g0w = win(fg0, PCOLS, FGC, 3)
                f1w = win(fg1, 0, 1036, 2)
                g1w = win(fg1, PCOLS, 1036, 2)
                h0w = win(h0, 0, PCOLS, 3)
                h1w = win(h1, 0, PCOLS, 2)

                wt_e = P[:, 0, 0:5:2, :]
                wt_o = P[:, 0, 1:5:2, :]

                # DVE products, all fused even/odd tap groups
                nc.vector.tensor_mul(P[:, 1, 0:5:2, :], f0w, wt_e)
                nc.vector.tensor_mul(P[:, 3, 0:5:2, :], g0w, wt_e)
                nc.vector.tensor_mul(P[:, 5, 0:5:2, :], h0w, wt_e)
                nc.vector.tensor_mul(P[:, 2, 0:5:2, :], P[:, 1, 0:5:2, :], f0w)
                nc.vector.tensor_mul(P[:, 4, 0:5:2, :], P[:, 3, 0:5:2, :], g0w)
                nc.vector.tensor_mul(P[:, 1, 1:5:2, :], f1w, wt_o)
                nc.vector.tensor_mul(P[:, 3, 1:5:2, :], g1w, wt_o)
                nc.vector.tensor_mul(P[:, 5, 1:5:2, :], h1w, wt_o)
                nc.vector.tensor_mul(P[:, 2, 1:5:2, :], P[:, 1, 1:5:2, :], f1w)
                nc.vector.tensor_mul(P[:, 4, 1:5:2, :], P[:, 3, 1:5:2, :], g1w)

                # PE: selector matmuls; group A (slots 0-2) then group B (3-5)
                if s == 0:
                    psA = psum_stats.tile([125, 3, 512], F32, tag="psA", name="psA")
                    psB = psum_stats.tile([125, 3, 512], F32, tag="psB", name="psB")
                border = [0, 2, 4, 1, 3]
                for grp, ps in ((0, psA), (1, psB)):
                    for j in range(3):
                        slot = grp * 3 + j
                        for i, b in enumerate(border):
                            mov = bass.AP(
                                tensor=P.tensor,
                                offset=slot * 2560 + b * 512,
                                ap=[[6 * 2560, 125], [1, 512]],
                            )
                            nc.tensor.matmul(
                                ps[:, j, :],
                                st_all[:, s, :],
                                mov,
                                start=(s == 0 and i == 0),
                                stop=(s == last_s and i == 4),
                            )

                if s == last_s:
                    p = 125 if g < 2 else 25
                    gst = gstp.tile([125, 6, 512], BF16, tag="gst")
                    # A copied alone so its row-sums ride along for free
                    nc.scalar.activation(
                        gst[:, 0, :], psA[:, 0, :], AF.Copy,
                        accum_out=accA[g][:],
                    )
                    # squares of the means, straight from PSUM
                    t2 = epi.tile([125, 512], BF16, tag="t2", name="t2")
                    nc.scalar.square(t2[:p, :], psA[:p, 1, :])
                    t3 = epi.tile([125, 512], BF16, tag="t3", name="t3")
                    nc.scalar.square(t3[:p, :], psB[:p, 0, :])
                    nc.scalar.copy(gst[:, 1:3, :], psA[:, 1:3, :])
                    nc.scalar.copy(gst[:, 3:6, :], psB[:])

                    A = gst[:p, 0, :]
                    Cf = gst[:p, 2, :]
                    Cg = gst[:p, 4, :]
                    Bh = gst[:p, 5, :]

                    def et(tag, dt=BF16):
                        return epi.tile([125, 512], dt, tag=tag, name=tag)

                    # DVE: variances  vf = max(Cf*A - Bf^2, 0) + eps*A^2
                    e = et("e")
                    nc.vector.scalar_tensor_tensor(
                        e[:p, :], A, EPS, A, op0=ALU.mult, op1=ALU.mult
                    )
                    t1 = et("t1")
                    nc.vector.tensor_mul(t1[:p, :], Cf, A)
                    xf = et("xf")
                    nc.vector.scalar_tensor_tensor(
                        xf[:p, :], t2[:p, :], -1.0, t1[:p, :],
                        op0=ALU.mult, op1=ALU.add,
                    )
                    vf = et("vf")
                    nc.vector.scalar_tensor_tensor(
                        vf[:p, :], xf[:p, :], 0.0, e[:p, :],
                        op0=ALU.max, op1=ALU.add,
                    )
                    t1g = et("t1g")
                    nc.vector.tensor_mul(t1g[:p, :], Cg, A)
                    xg = et("xg")
                    nc.vector.scalar_tensor_tensor(
                        xg[:p, :], t3[:p, :], -1.0, t1g[:p, :],
                        op0=ALU.mult, op1=ALU.add,
                    )
                    vg = et("vg")
                    nc.vector.scalar_tensor_tensor(
                        vg[:p, :], xg[:p, :], 0.0, e[:p, :],
                        op0=ALU.max, op1=ALU.add,
                    )

                    # ScalarE: sf = sqrt(vf) = A*std_fake; sg = K*A*std_gamma
                    sf = et("sf")
                    nc.scalar.activation(sf[:p, :], vf[:p, :], AF.Sqrt)
                    sg = et("sg")
                    nc.scalar.activation(
                        sg[:p, :], vg[:p, :], AF.Sqrt, scale=k2_sb[:p, :]
                    )

                    # bh2 = Bh + eps*A (DVE), num/den0 on GpSimd
                    bh2 = et("bh2")
                    nc.vector.scalar_tensor_tensor(
                        bh2[:p, :], A, EPS, Bh, op0=ALU.mult, op1=ALU.add
                    )
                    num = et("num")
                    nc.gpsimd.tensor_mul(num[:p, :], sg[:p, :], bh2[:p, :])
                    den0 = et("den0", F32)
                    nc.gpsimd.tensor_mul(den0[:p, :], A, sf[:p, :])

                    # DVE: r = num / (den0 + num), contrib = r*(A-1)
                    den = et("den", F32)
                    nc.vector.scalar_tensor_tensor(
                        den[:p, :], den0[:p, :], 1e-30, num[:p, :],
                        op0=ALU.add, op1=ALU.add,
                    )
                    rden = et("rden", F32)
                    nc.vector.reciprocal_approx_fast(rden[:p, :], den[:p, :])
                    r = et("r")
                    nc.vector.tensor_mul(r[:p, :], num[:p, :], rden[:p, :])
                    cb = et("cb")
                    nc.vector.scalar_tensor_tensor(
                        cb[:p, :], A, -1.0, r[:p, :],
                        op0=ALU.add, op1=ALU.mult, accum_out=accC[g][:p, :],
                    )

            # ---------- final reduce ----------
            red = singles.tile([125, 2], F32)
            nc.vector.tensor_add(red[:, 0:1], accC[0][:], accC[1][:])
            nc.vector.tensor_add(red[0:25, 0:1], red[0:25, 0:1], accC[2][0:25, :])
            nc.vector.tensor_add(red[:, 1:2], accA[0][:], accA[1][:])
            nc.vector.tensor_add(red[:, 1:2], red[:, 1:2], accA[2][:])
            nc.sync.dma_start(out=out[:], in_=red[:])

    nc.compile()
    return nc


def _host_inputs(fake, gamma_hdr, hdr_original_im, r_weights, f_factors,
                 hdr_original_gray):
    """Build the 8 per-core input dicts (all image data host-cast to bf16)."""
    bf16 = ml_dtypes.bfloat16
    stat_np = np.zeros((5, 125, 125), dtype=np.float32)
    for s in range(5):
        for a in range(5):
            for q in range(25):
                stat_np[s, a * 25 + q, s * 25 + q] = 1.0
    stat_np = stat_np.astype(bf16)

    def padimg(x, cval):
        return np.pad(x, ((2, 22), (2, 2)), constant_values=cval).astype(
            np.float32
        )

    in_maps = []
    for c in range(N_CORES):
        b = c // 2
        r0 = (c % 2) * RPC
        slab = np.full((5, 5, VROWS, W_IMG), WPAD, dtype=np.float32)
        slab[:, :, :RPC, :] = r_weights[b, :, r0 : r0 + RPC, :].reshape(
            5, 5, RPC, W_IMG
        )
        slab = np.ascontiguousarray(slab.transpose(0, 2, 1, 3))  # [a, r, b, c]

        pf = padimg(fake[b, 0], 0.0)[r0 : r0 + PROWS]
        pg = padimg(gamma_hdr[b, 0], 0.0)[r0 : r0 + PROWS]
        imfg = np.ascontiguousarray(
            np.stack([pf, pg], axis=1).astype(bf16)
        )  # [280, 2, 516]
        ph = padimg(hdr_original_im[b, 0], 1.0)[r0 : r0 + PROWS]
        gidx = r0 + np.arange(PROWS)
        hm = ((gidx >= 2) & (gidx <= 513)).astype(np.float32).reshape(PROWS, 1)

        f = float(f_factors[b])
        scal = np.array([[1.0 - f, 1.0 / f, 0.0, 0.0]], dtype=np.float32)

        in_maps.append(
            {
                "wslab": slab.astype(bf16),
                "imfg": imfg,
                "imh": np.ascontiguousarray(ph).astype(bf16),
                "hmask": hm,
                "gray": np.ascontiguousarray(hdr_original_gray[b, 0]).astype(bf16),
                "scal": scal,
                "stat": stat_np,
            }
        )
    return in_maps


def kernel_run(inputs, **spmd_kwargs):
    """Returns (scalar_result, BassKernelResults)."""
    if "nc" not in _CACHE:
        _CACHE["nc"] = _build_nc()
    nc = _CACHE["nc"]
    in_maps = _host_inputs(**inputs)
    res = run_bass_kernel_spmd(nc, in_maps, list(range(N_CORES)), **spmd_kwargs)
    s1 = 0.0
    s2 = 0.0
    for r in res.results:
        o = np.asarray(r["out"], dtype=np.float64)
        s1 += o[:, 0].sum()
        s2 += o[:, 1].sum() - 512.0 * VROWS
    return np.float32(s1 / s2), res


def kernel(**inputs):
    result, _ = kernel_run(inputs)
    return result
